# revision 19
# baseline (speedup 1.0000x reference)
"""Trainium2 Bass kernel for the GNN message function.

Computes, for batch of graphs:
    out[b, 0:128,  n] = relu(W_e @ e_vw[b, :, n] + b_e)
    out[b, 128:256,n] = relu(W_h @ h_w[b, :, n] + b_h)

Sharding: data-parallel over the batch axis (32 batches -> 4 per core x 8
cores). The tiny Linear weights are replicated to every core.

Per-core kernel: for each of the 4 local batches, load e_vw[b]/h_w[b]
([256, 2048] fp32, 2 MiB each) into SBUF as [128, 2x2048] tiles (K-chunks
side by side), run 2 matmuls per 512-wide node tile accumulating over the
two K=128 chunks in PSUM, then a fused bias+ReLU on the scalar engine into
an SBUF output tile, and store [128, 2x2048] back to DRAM. Memory bound:
24 MiB of DMA per core vs ~55 us of fp32 PE work.
"""

import numpy as np

B, F, N = 32, 256, 2048   # batch, feature, nodes (fixed problem shape)
HALF = 128                # message_size // 2
NCORES = 8
BPC = B // NCORES         # batches per core
NT = 512                  # matmul moving free-dim tile (one PSUM bank)

# dtype mode for the matmul inputs: "fp32" (exact, 4 cyc/row) or
# "fp32r" (single-pass fp32, 1 cyc/row at N>=256)
MM_DTYPE = "fp32"

_CACHE = {}


def _build_nc(repeat=1):
    import concourse.mybir as mybir
    from concourse import bacc
    from concourse.tile import TileContext

    f32 = mybir.dt.float32
    mm_dt = mybir.dt.float32r if MM_DTYPE == "fp32r" else f32
    relu = mybir.ActivationFunctionType.Relu

    nc = bacc.Bacc("TRN2", target_bir_lowering=False, debug=False,
                   num_devices=NCORES)
    e = nc.dram_tensor("e_vw", [BPC, F, N], f32, kind="ExternalInput")
    h = nc.dram_tensor("h_w", [BPC, F, N], f32, kind="ExternalInput")
    # wT[li] = W_li.T  ([K=256, M=128]); li=0 -> edge linear, 1 -> node linear
    wT = nc.dram_tensor("wT", [2, F, HALF], f32, kind="ExternalInput")
    bias = nc.dram_tensor("bias", [2, HALF, 1], f32, kind="ExternalInput")
    out = nc.dram_tensor("out", [BPC, 2 * HALF, N], f32, kind="ExternalOutput")

    with TileContext(nc) as tc:
        with tc.tile_pool(name="const", bufs=1) as cpool, \
             tc.tile_pool(name="x", bufs=10) as xpool, \
             tc.tile_pool(name="o", bufs=4) as opool, \
             tc.tile_pool(name="ps", bufs=8, space="PSUM") as pspool:
            # Weights: one [128, 256] tile per linear; columns kc*128..
            # hold K-chunk kc of W^T (lhsT layout: [K=128 part, M=128 free]).
            # PE warm-up: dummy matmuls on a zeroed scratch tile fill the
            # dead window while the first loads land, so the tensor engine
            # is at full clock when real matmuls start (HAM ramp ~3us).
            warm = cpool.tile([128, NT], f32, tag="warm")
            nc.gpsimd.memset(warm[:, :], 0.0)
            for _ in range(6):
                wps = pspool.tile([128, NT], f32, tag="ps")
                nc.tensor.matmul(wps[:, :], warm[:, 0:128], warm[:, :],
                                 start=True, stop=True)

            # Constants go on the gpsimd (SWDGE) ring so the sync-engine
            # HWDGE ring starts streaming activations immediately.
            w_tiles = []
            b_tiles = []
            for li in range(2):
                wt = cpool.tile([128, F], f32, tag=f"w{li}")
                nc.gpsimd.dma_start(
                    out=wt.rearrange("p (c m) -> p c m", c=2),
                    in_=wT[li].rearrange("(c p) m -> p c m", p=128))
                w_tiles.append(wt)
                bt = cpool.tile([HALF, 1], f32, tag=f"b{li}")
                nc.gpsimd.dma_start(out=bt, in_=bias[li])
                b_tiles.append(bt)

            for b in [b for _ in range(repeat) for b in range(BPC)]:
                # 1 MiB load per (linear, K-chunk), in consumption order so
                # the first matmul starts after the first chunk lands.
                xs = {}
                for li, src in ((0, e), (1, h)):
                    for kc in range(2):
                        xt = xpool.tile([128, N], f32, tag="x")
                        nc.sync.dma_start(
                            out=xt, in_=src[b, kc * 128:(kc + 1) * 128, :])
                        xs[li, kc] = xt
                for li in range(2):
                    lhs0 = w_tiles[li][:, 0:HALF].bitcast(mm_dt)
                    lhs1 = w_tiles[li][:, HALF:2 * HALF].bitcast(mm_dt)
                    oh = opool.tile([128, N], f32, tag="o")
                    for t in range(N // NT):
                        sl = slice(t * NT, (t + 1) * NT)
                        ps = pspool.tile([128, NT], f32, tag="ps")
                        nc.tensor.matmul(ps[:, :], lhs0,
                                         xs[li, 0][:, sl].bitcast(mm_dt),
                                         start=True, stop=False)
                        nc.tensor.matmul(ps[:, :], lhs1,
                                         xs[li, 1][:, sl].bitcast(mm_dt),
                                         start=False, stop=True)
                        nc.scalar.activation(
                            out=oh[:, sl], in_=ps[:, :], func=relu,
                            bias=b_tiles[li])
                    # Stores go on the scalar engine's HWDGE ring: keeps the
                    # sync-engine FIFO loads-only (no head-of-line blocking
                    # of prefetches behind a store waiting on compute).
                    # Final batch: store in halves so the last piece (after
                    # the final activation) is small -> shorter tail.
                    orow = out[b, li * HALF:(li + 1) * HALF, :]
                    if b == BPC - 1:
                        nc.scalar.dma_start(out=orow[:, 0:N // 2],
                                            in_=oh[:, 0:N // 2])
                        nc.scalar.dma_start(out=orow[:, N // 2:N],
                                            in_=oh[:, N // 2:N])
                    else:
                        nc.scalar.dma_start(out=orow, in_=oh)
    nc.finalize()
    return nc


def get_nc(repeat=1):
    key = ("nc", repeat)
    if key not in _CACHE:
        _CACHE[key] = _build_nc(repeat)
    return _CACHE[key]


def make_in_maps(h_w, e_vw, W_e, b_e, W_h, b_h):
    """Shard the full inputs into per-core input maps."""
    wT = np.ascontiguousarray(
        np.stack([W_e.T, W_h.T]).astype(np.float32))            # [2, 256, 128]
    bias = np.ascontiguousarray(
        np.stack([b_e, b_h]).astype(np.float32)[:, :, None])    # [2, 128, 1]
    in_maps = []
    for c in range(NCORES):
        sl = slice(c * BPC, (c + 1) * BPC)
        in_maps.append({
            "e_vw": np.ascontiguousarray(e_vw[sl], dtype=np.float32),
            "h_w": np.ascontiguousarray(h_w[sl], dtype=np.float32),
            "wT": wT,
            "bias": bias,
        })
    return in_maps


def kernel(h_w, e_vw, W_e, b_e, W_h, b_h):
    from concourse.bass_utils import run_bass_kernel_spmd

    nc = get_nc()
    in_maps = make_in_maps(h_w, e_vw, W_e, b_e, W_h, b_h)
    res = run_bass_kernel_spmd(nc, in_maps, core_ids=list(range(NCORES)))
    return np.concatenate([r["out"] for r in res.results], axis=0)
